# revision 4
# baseline (speedup 1.0000x reference)
"""Trainium2 Bass kernel for the tied-embedding LSTM LM loss.

Structure (per the vocab-tensor-parallel sharding):
  Phase A: XW = emb[x] @ W_ih  for all (t,b) pairs        -- replicated
  Phase B: 128-step LSTM recurrence (g = XW_t + h_t @ W_hh) -- replicated
  Phase C: OUT.T = Wr @ H2.T ; logits = OUT @ emb_shard.T  -- vocab-sharded
           per-row sum(exp(logit)) partials + target-logit dots
  Host:    combine 8 sumexp partials, log-sum-exp, mask, reduce to scalar.

All matmuls run in bf16 (fp32 PSUM accumulation); LSTM cell state is fp32.
"""

import numpy as np
import ml_dtypes

import concourse.bass as bass
import concourse.bacc as bacc
import concourse.mybir as mybir
import concourse.tile as tile
from concourse.bass_utils import run_bass_kernel_spmd

FP32 = mybir.dt.float32
BF16 = mybir.dt.bfloat16
AF = mybir.ActivationFunctionType
ALU = mybir.AluOpType

V, E, H = 32000, 1024, 1024
T1, B = 129, 64
TX = T1 - 1               # 128 recurrence steps
R = TX * B                # 8192 (t,b) rows
NC = 8                    # cores
VS = V // NC              # 4000 vocab shard
KC = E // 128             # 8 contraction chunks
MC = R // 128             # 64 row chunks
NBLK = 16                 # 512-wide OUT.T column blocks
BW = R // NBLK            # 512


def build_program():
    nc = bacc.Bacc("TRN2", target_bir_lowering=False)

    # ---- inputs (per-core layouts prepared on host) ----
    xt = nc.dram_tensor("xt", [MC, 128, KC, 128], BF16, kind="ExternalInput")
    wih = nc.dram_tensor("wih", [128, KC, 4 * H], BF16, kind="ExternalInput")
    whh = nc.dram_tensor("whh", [128, KC, 4 * H], BF16, kind="ExternalInput")
    wrt = nc.dram_tensor("wrt", [128, KC, E], BF16, kind="ExternalInput")
    embt = nc.dram_tensor("embt", [128, KC, VS], BF16, kind="ExternalInput")
    eyt = nc.dram_tensor("eyt", [128, KC, R], BF16, kind="ExternalInput")
    ident = nc.dram_tensor("ident", [64, 64], BF16, kind="ExternalInput")
    ones128 = nc.dram_tensor("ones128", [128, 1], BF16, kind="ExternalInput")

    # ---- outputs ----
    s_out = nc.dram_tensor("s_out", [128, MC], FP32, kind="ExternalOutput")
    t_out = nc.dram_tensor("t_out", [NBLK, BW], FP32, kind="ExternalOutput")

    # ---- DRAM scratch ----
    xw_d = nc.dram_tensor("xw_d", [MC, 128, 4 * H], BF16, kind="Internal")
    h2t_d = nc.dram_tensor("h2t_d", [128, KC, R], BF16, kind="Internal")

    with tile.TileContext(nc) as tc:
        with (
            tc.tile_pool(name="psum", bufs=2, space="PSUM") as pp,
            tc.tile_pool(name="small", bufs=1) as smp,
        ):
            id_sb = smp.tile([64, 64], BF16, tag="id")
            nc.sync.dma_start(id_sb[:], ident[:])
            ones_sb = smp.tile([128, 1], BF16, tag="ones")
            nc.sync.dma_start(ones_sb[:], ones128[:])
            s_sb = smp.tile([128, MC], FP32, tag="s")

            # ================= Phase A: XW = X @ W_ih =================
            with (
                tc.tile_pool(name="wih_p", bufs=1) as wih_p,
                tc.tile_pool(name="a_io", bufs=3) as a_io,
            ):
                wih_sb = wih_p.tile([128, KC, 4 * H], BF16, tag="w")
                nc.sync.dma_start(wih_sb[:], wih[:])
                for mc in range(MC):
                    xt_sb = a_io.tile([128, KC, 128], BF16, tag="xt")
                    nc.sync.dma_start(xt_sb[:], xt[mc])
                    for hf in range(2):
                        ps = pp.tile([128, 2048], FP32, tag="ps")
                        for k in range(KC):
                            for nn in range(4):
                                nc.tensor.matmul(
                                    ps[:, nn * 512:(nn + 1) * 512],
                                    lhsT=xt_sb[:, k, :],
                                    rhs=wih_sb[:, k, hf * 2048 + nn * 512:
                                               hf * 2048 + (nn + 1) * 512],
                                    start=(k == 0), stop=(k == KC - 1),
                                )
                        xw_sb = a_io.tile([128, 2048], BF16, tag="xw")
                        nc.any.tensor_copy(xw_sb[:], ps[:])
                        nc.sync.dma_start(
                            xw_d[mc, :, hf * 2048:(hf + 1) * 2048], xw_sb[:])

            # ================= Phase B: LSTM recurrence =================
            with (
                tc.tile_pool(name="whh_p", bufs=1) as whh_p,
                tc.tile_pool(name="b_io", bufs=2) as b_io,
                tc.tile_pool(name="b_st", bufs=2) as b_st,
            ):
                whh_sb = whh_p.tile([128, KC, 4 * H], BF16, tag="w")
                nc.sync.dma_start(whh_sb[:], whh[:])

                ht_sb = b_st.tile([128, KC, 64], BF16, tag="ht")
                ct_sb = b_st.tile([64, H], FP32, tag="ct")
                nc.any.memset(ht_sb[:], 0.0)
                nc.any.memset(ct_sb[:], 0.0)

                for t in range(TX):
                    xwb = b_io.tile([64, 4 * H], BF16, tag="xwb")
                    nc.sync.dma_start(
                        xwb[:], xw_d[t // 2, (t % 2) * 64:(t % 2) * 64 + 64, :])

                    ghalf = []
                    for hf in range(2):
                        g = pp.tile([64, 2048], FP32, tag="ps")
                        for nn in range(4):
                            nc.tensor.matmul(
                                g[:, nn * 512:(nn + 1) * 512],
                                lhsT=id_sb[:],
                                rhs=xwb[:, hf * 2048 + nn * 512:
                                        hf * 2048 + (nn + 1) * 512],
                                start=True, stop=False,
                            )
                        for k in range(KC):
                            for nn in range(4):
                                nc.tensor.matmul(
                                    g[:, nn * 512:(nn + 1) * 512],
                                    lhsT=ht_sb[:, k, :],
                                    rhs=whh_sb[:, k, hf * 2048 + nn * 512:
                                               hf * 2048 + (nn + 1) * 512],
                                    start=False, stop=(k == KC - 1),
                                )
                        ghalf.append(g)

                    gates = b_io.tile([64, 4 * H], FP32, tag="gates")
                    # layout: [i | f] from half0, [gg | o] from half1
                    nc.scalar.activation(gates[:, 0:2048], ghalf[0][:, 0:2048],
                                         AF.Sigmoid)
                    nc.scalar.activation(gates[:, 2048:3072], ghalf[1][:, 0:1024],
                                         AF.Tanh)
                    nc.scalar.activation(gates[:, 3072:4096], ghalf[1][:, 1024:2048],
                                         AF.Sigmoid)

                    t1 = b_io.tile([64, H], FP32, tag="t1")
                    nc.vector.tensor_tensor(t1[:], gates[:, 0:1024],
                                            gates[:, 2048:3072], op=ALU.mult)
                    t2 = b_io.tile([64, H], FP32, tag="t2")
                    nc.vector.tensor_tensor(t2[:], gates[:, 1024:2048],
                                            ct_sb[:], op=ALU.mult)
                    cn = b_st.tile([64, H], FP32, tag="ct")
                    nc.vector.tensor_tensor(cn[:], t1[:], t2[:], op=ALU.add)
                    tn = b_io.tile([64, H], FP32, tag="tn")
                    nc.scalar.activation(tn[:], cn[:], AF.Tanh)
                    hn = b_io.tile([64, H], BF16, tag="hn")
                    nc.vector.tensor_tensor(hn[:], gates[:, 3072:4096], tn[:],
                                            op=ALU.mult)
                    ct_sb = cn

                    trp = pp.tile([128, 512], BF16, tag="ps")
                    for k in range(KC):
                        nc.tensor.transpose(
                            trp[:, k * 64:(k + 1) * 64],
                            hn[:, k * 128:(k + 1) * 128], id_sb[:])
                    ht_sb = b_st.tile([128, KC, 64], BF16, tag="ht")
                    nc.any.tensor_copy(ht_sb[:], trp[:])
                    nc.sync.dma_start(h2t_d[:, :, t * 64:(t + 1) * 64], ht_sb[:])

            # ================= Phase C: readout + decoder =================
            with (
                tc.tile_pool(name="c_w", bufs=1) as c_w,
                tc.tile_pool(name="c_io", bufs=2) as c_io,
                tc.tile_pool(name="c_sc", bufs=2) as c_sc,
            ):
                wrt_sb = c_w.tile([128, KC, E], BF16, tag="wrt")
                nc.sync.dma_start(wrt_sb[:], wrt[:])
                embt_sb = c_w.tile([128, KC, VS], BF16, tag="embt")
                nc.sync.dma_start(embt_sb[:], embt[:])

                for nb in range(NBLK):
                    h2b = c_io.tile([128, KC, BW], BF16, tag="h2b")
                    nc.sync.dma_start(h2b[:], h2t_d[:, :, nb * BW:(nb + 1) * BW])

                    outt = c_io.tile([128, KC, BW], BF16, tag="outt")
                    for m in range(KC):
                        ps1 = pp.tile([128, BW], FP32, tag="ps")
                        for k in range(KC):
                            nc.tensor.matmul(
                                ps1[:], lhsT=wrt_sb[:, k, m * 128:(m + 1) * 128],
                                rhs=h2b[:, k, :],
                                start=(k == 0), stop=(k == KC - 1))
                        nc.any.tensor_copy(outt[:, m, :], ps1[:])

                    # decoder: 4 row-chunks of 128 rows each
                    for mm in range(4):
                        gmc = nb * 4 + mm
                        sacc = c_sc.tile([128, 2], FP32, tag="sacc")
                        for hf in range(2):
                            ps2 = pp.tile([128, 2000], FP32, tag="ps")
                            for k in range(KC):
                                for nn in range(4):
                                    nc.tensor.matmul(
                                        ps2[:, nn * 500:(nn + 1) * 500],
                                        lhsT=outt[:, k, mm * 128:(mm + 1) * 128],
                                        rhs=embt_sb[:, k, hf * 2000 + nn * 500:
                                                    hf * 2000 + (nn + 1) * 500],
                                        start=(k == 0), stop=(k == KC - 1))
                            esc = c_sc.tile([128, 2000], BF16, tag="esc")
                            nc.scalar.activation(esc[:], ps2[:], AF.Exp,
                                                 accum_out=sacc[:, hf:hf + 1])
                        nc.vector.tensor_tensor(s_sb[:, gmc:gmc + 1],
                                                sacc[:, 0:1], sacc[:, 1:2],
                                                op=ALU.add)

                    # target-logit dots for these 512 rows (all cores redundant)
                    eyb = c_io.tile([128, KC, BW], BF16, tag="eyb")
                    nc.sync.dma_start(eyb[:], eyt[:, :, nb * BW:(nb + 1) * BW])
                    prod = c_io.tile([128, KC, BW], BF16, tag="prod")
                    nc.vector.tensor_tensor(prod[:], outt[:], eyb[:], op=ALU.mult)
                    tps = pp.tile([1, BW], FP32, tag="ps")
                    for k in range(KC):
                        nc.tensor.matmul(tps[:], lhsT=ones_sb[:], rhs=prod[:, k, :],
                                         start=(k == 0), stop=(k == KC - 1))
                    tsb = c_sc.tile([1, BW], FP32, tag="tsb")
                    nc.any.tensor_copy(tsb[:], tps[:])
                    nc.sync.dma_start(t_out[nb:nb + 1, :], tsb[:])

            nc.sync.dma_start(s_out[:], s_sb[:])

    nc.compile()
    return nc


_PROGRAM = None


def _get_program():
    global _PROGRAM
    if _PROGRAM is None:
        _PROGRAM = build_program()
    return _PROGRAM


def _prep_inputs(data, mask, emb, W_ih, W_hh, b, Wr, br, bd):
    assert not np.any(b) and not np.any(br), "nonzero LSTM/readout bias unsupported"
    bf = ml_dtypes.bfloat16
    x = np.ascontiguousarray(data[:-1]).astype(np.int64).reshape(-1)
    y = np.ascontiguousarray(data[1:]).astype(np.int64).reshape(-1)

    X = emb[x]                                    # [R, E] fp32
    # xt[mc, p, k, m] = X[mc*128 + m, k*128 + p]
    xt = np.ascontiguousarray(
        X.reshape(MC, 128, KC, 128).transpose(0, 3, 2, 1)).astype(bf)
    wih = np.ascontiguousarray(
        W_ih.reshape(KC, 128, 4 * H).transpose(1, 0, 2)).astype(bf)
    whh = np.ascontiguousarray(
        W_hh.reshape(KC, 128, 4 * H).transpose(1, 0, 2)).astype(bf)
    # wrt[p, k, e] = Wr[e, k*128 + p]
    wrt = np.ascontiguousarray(
        Wr.T.reshape(KC, 128, E).transpose(1, 0, 2)).astype(bf)
    EY = emb[y]                                   # [R, E]
    eyt = np.ascontiguousarray(
        EY.T.reshape(KC, 128, R).transpose(1, 0, 2)).astype(bf)
    ident = np.eye(64, dtype=bf)
    ones = np.ones((128, 1), dtype=bf)

    in_maps = []
    for j in range(NC):
        shard = emb[j * VS:(j + 1) * VS]          # [VS, E]
        embt = np.ascontiguousarray(
            shard.T.reshape(KC, 128, VS).transpose(1, 0, 2)).astype(bf)
        in_maps.append({
            "xt": xt, "wih": wih, "whh": whh, "wrt": wrt,
            "embt": embt, "eyt": eyt, "ident": ident, "ones128": ones,
        })
    return in_maps, y


def _combine(results, y, mask, bd):
    S = np.zeros(R, np.float64)
    for j in range(NC):
        # s_out[p, mc] -> row mc*128 + p
        S += results[j]["s_out"].T.reshape(-1).astype(np.float64)
    Tt = results[0]["t_out"].reshape(-1).astype(np.float64) + bd[y]
    m = mask[1:].reshape(-1).astype(np.float64)
    nll = np.log(S) - Tt
    loss = (nll * m).sum() / (B * B)
    return np.float32(loss)


def _run(in_maps, **kw):
    nc = _get_program()
    return run_bass_kernel_spmd(nc, in_maps, core_ids=list(range(NC)), **kw)


def kernel(data, mask, emb, W_ih, W_hh, b, Wr, br, bd):
    data = np.asarray(data)
    mask = np.asarray(mask).astype(np.float32)
    emb = np.asarray(emb).astype(np.float32)
    args = dict(data=data, mask=mask, emb=emb,
                W_ih=np.asarray(W_ih, np.float32),
                W_hh=np.asarray(W_hh, np.float32),
                b=np.asarray(b, np.float32), Wr=np.asarray(Wr, np.float32),
                br=np.asarray(br, np.float32), bd=np.asarray(bd, np.float32))
    in_maps, y = _prep_inputs(**args)
    res = _run(in_maps)
    return _combine(res.results, y, mask, np.asarray(bd, np.float64))


# revision 8
# speedup vs baseline: 1.1521x; 1.1521x over previous
"""Trainium2 Bass kernel for the tied-embedding LSTM LM loss.

Structure (per the vocab-tensor-parallel sharding):
  Phase A: XW = emb[x] @ W_ih  for all (t,b) pairs        -- replicated
  Phase B: 128-step LSTM recurrence (g = XW_t + h_t @ W_hh) -- replicated
  Phase C: OUT.T = Wr @ H2.T ; logits = OUT @ emb_shard.T  -- vocab-sharded
           per-row sum(exp(logit)) partials + target-logit dots
  Host:    combine 8 sumexp partials, log-sum-exp, mask, reduce to scalar.

All matmuls run in bf16 (fp32 PSUM accumulation); LSTM cell state is fp32.
"""

import numpy as np
import ml_dtypes

import concourse.bass as bass
import concourse.bacc as bacc
import concourse.mybir as mybir
import concourse.tile as tile
from concourse.bass_utils import run_bass_kernel_spmd

FP32 = mybir.dt.float32
BF16 = mybir.dt.bfloat16
AF = mybir.ActivationFunctionType
ALU = mybir.AluOpType

V, E, H = 32000, 1024, 1024
T1, B = 129, 64
TX = T1 - 1               # 128 recurrence steps
R = TX * B                # 8192 (t,b) rows
NC = 8                    # cores
VS = V // NC              # 4000 vocab shard
KC = E // 128             # 8 contraction chunks
MC = R // 128             # 64 row chunks
NBLK = 16                 # 512-wide OUT.T column blocks
BW = R // NBLK            # 512


def build_program():
    nc = bacc.Bacc("TRN2", target_bir_lowering=False)

    # ---- inputs (per-core layouts prepared on host) ----
    xt = nc.dram_tensor("xt", [MC, 128, KC, 128], BF16, kind="ExternalInput")
    wih = nc.dram_tensor("wih", [128, KC, 4 * H], BF16, kind="ExternalInput")
    whh = nc.dram_tensor("whh", [128, KC, 4 * H], BF16, kind="ExternalInput")
    wrt = nc.dram_tensor("wrt", [128, KC, E], BF16, kind="ExternalInput")
    embt = nc.dram_tensor("embt", [128, KC, VS], BF16, kind="ExternalInput")
    eyt = nc.dram_tensor("eyt", [128, KC, R], BF16, kind="ExternalInput")
    ident = nc.dram_tensor("ident", [64, 64], BF16, kind="ExternalInput")
    ones128 = nc.dram_tensor("ones128", [128, 1], BF16, kind="ExternalInput")

    # ---- outputs ----
    s_out = nc.dram_tensor("s_out", [128, MC], FP32, kind="ExternalOutput")
    t_out = nc.dram_tensor("t_out", [NBLK, BW], FP32, kind="ExternalOutput")

    # ---- DRAM scratch ----
    xw_d = nc.dram_tensor("xw_d", [MC, 128, 4 * H], BF16, kind="Internal")
    outt_d = nc.dram_tensor("outt_d", [128, KC, R], BF16, kind="Internal")

    with tile.TileContext(nc) as tc:
        with (
            tc.tile_pool(name="psum", bufs=2, space="PSUM") as pp,
            tc.tile_pool(name="small", bufs=1) as smp,
        ):
            id_sb = smp.tile([64, 64], BF16, tag="id")
            nc.sync.dma_start(id_sb[:], ident[:])
            ones_sb = smp.tile([128, 1], BF16, tag="ones")
            nc.sync.dma_start(ones_sb[:], ones128[:])
            s_sb = smp.tile([128, MC], FP32, tag="s")

            # ================= Phase A: XW = X @ W_ih =================
            with (
                tc.tile_pool(name="wih_p", bufs=1) as wih_p,
                tc.tile_pool(name="a_io", bufs=3) as a_io,
            ):
                wih_sb = wih_p.tile([128, KC, 4 * H], BF16, tag="w")
                nc.sync.dma_start(wih_sb[:], wih[:])
                for mc in range(MC):
                    xt_sb = a_io.tile([128, KC, 128], BF16, tag="xt")
                    nc.sync.dma_start(xt_sb[:], xt[mc])
                    for hf in range(2):
                        ps = pp.tile([128, 2048], FP32, tag="ps")
                        for k in range(KC):
                            for nn in range(4):
                                nc.tensor.matmul(
                                    ps[:, nn * 512:(nn + 1) * 512],
                                    lhsT=xt_sb[:, k, :],
                                    rhs=wih_sb[:, k, hf * 2048 + nn * 512:
                                               hf * 2048 + (nn + 1) * 512],
                                    start=(k == 0), stop=(k == KC - 1),
                                )
                        xw_sb = a_io.tile([128, 2048], BF16, tag="xw")
                        nc.any.tensor_copy(xw_sb[:], ps[:])
                        nc.sync.dma_start(
                            xw_d[mc, :, hf * 2048:(hf + 1) * 2048], xw_sb[:])

            # ================= Phase B: LSTM recurrence =================
            with (
                tc.tile_pool(name="whh_p", bufs=1) as whh_p,
                tc.tile_pool(name="b_io", bufs=2) as b_io,
                tc.tile_pool(name="b_st", bufs=2) as b_st,
            ):
                whh_sb = whh_p.tile([128, KC, 4 * H], BF16, tag="w")
                nc.sync.dma_start(whh_sb[:], whh[:])
                wrt_sb = whh_p.tile([128, KC, E], BF16, tag="wrt")
                nc.sync.dma_start(wrt_sb[:], wrt[:])

                ht_sb = b_st.tile([128, KC, 64], BF16, tag="ht")
                ct_sb = b_st.tile([64, H], FP32, tag="ct")
                nc.any.memset(ht_sb[:], 0.0)
                nc.any.memset(ct_sb[:], 0.0)

                for t in range(TX):
                    xwb = b_io.tile([64, 4 * H], BF16, tag="xwb")
                    nc.sync.dma_start(
                        xwb[:], xw_d[t // 2, (t % 2) * 64:(t % 2) * 64 + 64, :])

                    ghalf = []
                    for hf in range(2):
                        g = pp.tile([64, 2048], FP32, tag="ps")
                        for nn in range(4):
                            nc.tensor.matmul(
                                g[:, nn * 512:(nn + 1) * 512],
                                lhsT=id_sb[:],
                                rhs=xwb[:, hf * 2048 + nn * 512:
                                        hf * 2048 + (nn + 1) * 512],
                                start=True, stop=False,
                            )
                        for k in range(KC):
                            for nn in range(4):
                                nc.tensor.matmul(
                                    g[:, nn * 512:(nn + 1) * 512],
                                    lhsT=ht_sb[:, k, :],
                                    rhs=whh_sb[:, k, hf * 2048 + nn * 512:
                                               hf * 2048 + (nn + 1) * 512],
                                    start=False, stop=(k == KC - 1),
                                )
                        ghalf.append(g)

                    gates = b_io.tile([64, 4 * H], FP32, tag="gates")
                    # layout: [i | f] from half0, [gg | o] from half1
                    nc.scalar.activation(gates[:, 0:2048], ghalf[0][:, 0:2048],
                                         AF.Sigmoid)
                    nc.scalar.activation(gates[:, 2048:3072], ghalf[1][:, 0:1024],
                                         AF.Tanh)
                    nc.scalar.activation(gates[:, 3072:4096], ghalf[1][:, 1024:2048],
                                         AF.Sigmoid)

                    t1 = b_io.tile([64, H], FP32, tag="t1")
                    nc.vector.tensor_tensor(t1[:], gates[:, 0:1024],
                                            gates[:, 2048:3072], op=ALU.mult)
                    t2 = b_io.tile([64, H], FP32, tag="t2")
                    nc.vector.tensor_tensor(t2[:], gates[:, 1024:2048],
                                            ct_sb[:], op=ALU.mult)
                    cn = b_st.tile([64, H], FP32, tag="ct")
                    nc.vector.tensor_tensor(cn[:], t1[:], t2[:], op=ALU.add)
                    tn = b_io.tile([64, H], FP32, tag="tn")
                    nc.scalar.activation(tn[:], cn[:], AF.Tanh)
                    hn = b_io.tile([64, H], BF16, tag="hn")
                    nc.vector.tensor_tensor(hn[:], gates[:, 3072:4096], tn[:],
                                            op=ALU.mult)
                    ct_sb = cn

                    trp = pp.tile([128, 512], BF16, tag="ps")
                    for k in range(KC):
                        nc.tensor.transpose(
                            trp[:, k * 64:(k + 1) * 64],
                            hn[:, k * 128:(k + 1) * 128], id_sb[:])
                    ht_sb = b_st.tile([128, KC, 64], BF16, tag="ht")
                    nc.any.tensor_copy(ht_sb[:], trp[:])

                    # readout OUT.T columns for this step -- fills the PE
                    # idle tail (keeps HAM warm) and removes phase-C1
                    rop = pp.tile([128, 512], FP32, tag="ps")
                    for m in range(KC):
                        for k in range(KC):
                            nc.tensor.matmul(
                                rop[:, m * 64:(m + 1) * 64],
                                lhsT=wrt_sb[:, k, m * 128:(m + 1) * 128],
                                rhs=ht_sb[:, k, :],
                                start=(k == 0), stop=(k == KC - 1))
                    ro_sb = b_io.tile([128, KC, 64], BF16, tag="ro")
                    nc.any.tensor_copy(ro_sb[:], rop[:])
                    nc.sync.dma_start(outt_d[:, :, t * 64:(t + 1) * 64], ro_sb[:])

            # ================= Phase C: readout + decoder =================
            with (
                tc.tile_pool(name="c_w", bufs=1) as c_w,
                tc.tile_pool(name="c_io", bufs=2) as c_io,
                tc.tile_pool(name="c_sc", bufs=2) as c_sc,
            ):
                embt_sb = c_w.tile([128, KC, VS], BF16, tag="embt")
                nc.sync.dma_start(embt_sb[:], embt[:])

                for nb in range(NBLK):
                    outt = c_io.tile([128, KC, BW], BF16, tag="outt")
                    nc.sync.dma_start(outt[:], outt_d[:, :, nb * BW:(nb + 1) * BW])

                    # decoder: 4 row-chunks of 128 rows each
                    for mm in range(4):
                        gmc = nb * 4 + mm
                        sacc = c_sc.tile([128, 2], FP32, tag="sacc")
                        for hf in range(2):
                            ps2 = pp.tile([128, 2000], FP32, tag="ps")
                            for k in range(KC):
                                for nn in range(4):
                                    nc.tensor.matmul(
                                        ps2[:, nn * 500:(nn + 1) * 500],
                                        lhsT=outt[:, k, mm * 128:(mm + 1) * 128],
                                        rhs=embt_sb[:, k, hf * 2000 + nn * 500:
                                                    hf * 2000 + (nn + 1) * 500],
                                        start=(k == 0), stop=(k == KC - 1))
                            esc = c_sc.tile([128, 2000], BF16, tag="esc")
                            nc.scalar.activation(esc[:], ps2[:], AF.Exp,
                                                 accum_out=sacc[:, hf:hf + 1])
                        nc.vector.tensor_tensor(s_sb[:, gmc:gmc + 1],
                                                sacc[:, 0:1], sacc[:, 1:2],
                                                op=ALU.add)

                    # target-logit dots for these 512 rows (all cores redundant)
                    eyb = c_io.tile([128, KC, BW], BF16, tag="eyb")
                    nc.sync.dma_start(eyb[:], eyt[:, :, nb * BW:(nb + 1) * BW])
                    prod = c_io.tile([128, KC, BW], BF16, tag="prod")
                    nc.vector.tensor_tensor(prod[:], outt[:], eyb[:], op=ALU.mult)
                    tps = pp.tile([1, BW], FP32, tag="ps")
                    for k in range(KC):
                        nc.tensor.matmul(tps[:], lhsT=ones_sb[:], rhs=prod[:, k, :],
                                         start=(k == 0), stop=(k == KC - 1))
                    tsb = c_sc.tile([1, BW], FP32, tag="tsb")
                    nc.any.tensor_copy(tsb[:], tps[:])
                    nc.sync.dma_start(t_out[nb:nb + 1, :], tsb[:])

            nc.sync.dma_start(s_out[:], s_sb[:])

    nc.compile()
    return nc


_PROGRAM = None


def _get_program():
    global _PROGRAM
    if _PROGRAM is None:
        _PROGRAM = build_program()
    return _PROGRAM


def _prep_inputs(data, mask, emb, W_ih, W_hh, b, Wr, br, bd):
    assert not np.any(b) and not np.any(br), "nonzero LSTM/readout bias unsupported"
    bf = ml_dtypes.bfloat16
    x = np.ascontiguousarray(data[:-1]).astype(np.int64).reshape(-1)
    y = np.ascontiguousarray(data[1:]).astype(np.int64).reshape(-1)

    X = emb[x]                                    # [R, E] fp32
    # xt[mc, p, k, m] = X[mc*128 + m, k*128 + p]
    xt = np.ascontiguousarray(
        X.reshape(MC, 128, KC, 128).transpose(0, 3, 2, 1)).astype(bf)
    wih = np.ascontiguousarray(
        W_ih.reshape(KC, 128, 4 * H).transpose(1, 0, 2)).astype(bf)
    whh = np.ascontiguousarray(
        W_hh.reshape(KC, 128, 4 * H).transpose(1, 0, 2)).astype(bf)
    # wrt[p, k, e] = Wr[e, k*128 + p]
    wrt = np.ascontiguousarray(
        Wr.T.reshape(KC, 128, E).transpose(1, 0, 2)).astype(bf)
    EY = emb[y]                                   # [R, E]
    eyt = np.ascontiguousarray(
        EY.T.reshape(KC, 128, R).transpose(1, 0, 2)).astype(bf)
    ident = np.eye(64, dtype=bf)
    ones = np.ones((128, 1), dtype=bf)

    in_maps = []
    for j in range(NC):
        shard = emb[j * VS:(j + 1) * VS]          # [VS, E]
        embt = np.ascontiguousarray(
            shard.T.reshape(KC, 128, VS).transpose(1, 0, 2)).astype(bf)
        in_maps.append({
            "xt": xt, "wih": wih, "whh": whh, "wrt": wrt,
            "embt": embt, "eyt": eyt, "ident": ident, "ones128": ones,
        })
    return in_maps, y


def _combine(results, y, mask, bd):
    S = np.zeros(R, np.float64)
    for j in range(NC):
        # s_out[p, mc] -> row mc*128 + p
        S += results[j]["s_out"].T.reshape(-1).astype(np.float64)
    Tt = results[0]["t_out"].reshape(-1).astype(np.float64) + bd[y]
    m = mask[1:].reshape(-1).astype(np.float64)
    nll = np.log(S) - Tt
    loss = (nll * m).sum() / (B * B)
    return np.float32(loss)


def _run(in_maps, **kw):
    nc = _get_program()
    return run_bass_kernel_spmd(nc, in_maps, core_ids=list(range(NC)), **kw)


def kernel(data, mask, emb, W_ih, W_hh, b, Wr, br, bd):
    data = np.asarray(data)
    mask = np.asarray(mask).astype(np.float32)
    emb = np.asarray(emb).astype(np.float32)
    args = dict(data=data, mask=mask, emb=emb,
                W_ih=np.asarray(W_ih, np.float32),
                W_hh=np.asarray(W_hh, np.float32),
                b=np.asarray(b, np.float32), Wr=np.asarray(Wr, np.float32),
                br=np.asarray(br, np.float32), bd=np.asarray(bd, np.float32))
    in_maps, y = _prep_inputs(**args)
    res = _run(in_maps)
    return _combine(res.results, y, mask, np.asarray(bd, np.float64))
